# revision 22
# baseline (speedup 1.0000x reference)
"""Category-specific linear (MoE-routing style) Trainium2 Bass kernel.

Computes out[n] = x[n] @ W[cat_ids[n]] + b[cat_ids[n]] for
x: [N, M, D_IN] f32, cat_ids: [N] int64, W: [C, D_IN, D_H] f32, b: [C, D_H] f32.

Strategy (8-core SPMD, full inputs in / full output out):
  Host: stable-sort samples by category, split into 8 equal shards of
  N/8 samples (perfect load balance).  Within a shard, each category is a
  contiguous run padded to whole 128-row tiles; runs are ordered
  longest-first so tiles [0, SP) of every core use weight slot 0 and can
  be emitted with a *static* weight index (no dependency on the index
  register load at startup).  x rows are pre-transposed on the host into
  [2, 128, NT*128] f16 so the contraction dim lands on SBUF partitions.
  Each core gets a deduplicated slot-major weight table (<=KMAX distinct
  categories) and a per-tile weight-slot index.
  Device: weight table in SBUF, slot 0 DMA'd first so the first matmul
  only waits on 128KB of W; remaining per-tile slots are selected with a
  dynamic slice whose index registers are loaded in two batched
  values_loads that hide behind already-queued matmul work.  Two
  accumulating matmuls per tile (contraction 256 = 2x128), PSUM -> SBUF
  f16 casts alternating between the DVE and Pool engines, batched stores
  on the Act ring with a tapered tail.
"""

import os
import sys

import numpy as np

for _p in ("/opt/trn_rl_repo",):
    if os.path.isdir(_p) and _p not in sys.path:
        sys.path.insert(0, _p)

import concourse.bass as bass  # noqa: E402
import concourse.mybir as mybir  # noqa: E402
import concourse.tile as tile  # noqa: E402
from concourse import bacc  # noqa: E402
from concourse.bass import ds  # noqa: E402
from concourse.bass_utils import run_bass_kernel_spmd  # noqa: E402

NCORES = 8
P = 128  # SBUF partitions / rows per tile
D_IN = 256  # contraction dim (2 chunks of 128)
D_H = 256  # output dim
ROWS_PER_SAMPLE = 16
SPT = P // ROWS_PER_SAMPLE  # samples per tile = 8
OB = 4  # tiles per psum group / cast
OS = 8  # tiles per out-store DMA
VLOAD = 16  # values per PE index-register load

# filled by kernel() for test harness introspection
last_results = None


def _pack(x, cat_ids, W):
    """Host-side routing: sort, shard, pad, transpose, dedup weights.

    Returns (in_maps, scatter_info, NT, KMAX, SP).
    scatter_info[k] = (sample_ids_per_padded_slot [NT*SPT] int64, valid mask)
    SP = static prefix: tiles [0, SP) use weight slot 0 on every core.
    """
    N, M, Din = x.shape
    assert M == ROWS_PER_SAMPLE and Din == D_IN
    assert N % NCORES == 0

    cat = np.asarray(cat_ids).astype(np.int64).ravel()
    order = np.argsort(cat, kind="stable")
    cats_sorted = cat[order]

    # global category runs over the sorted sample list
    bounds = np.flatnonzero(np.diff(cats_sorted)) + 1
    seg_starts = np.concatenate([[0], bounds])
    seg_ends = np.concatenate([bounds, [N]])
    segments = [
        (int(cats_sorted[s]), int(s), int(e))
        for s, e in zip(seg_starts, seg_ends)
    ]

    def pack(T):
        """Greedy-pack category runs into cores of <= T tiles each.

        A run cut mid-category always cuts at an SPT-sample multiple, so
        cuts cost no padding; only each core-local run tail pads to a tile.
        Returns per-core run lists [(cat, ids_padded_to_tiles)] or None.
        """
        cores = []
        cur_runs, used = [], 0
        rem = list(segments)
        i = 0

        def close():
            nonlocal cur_runs, used
            cores.append(cur_runs)
            cur_runs, used = [], 0

        while i < len(rem):
            c, s, e = rem[i]
            n = e - s
            tiles_need = (n + SPT - 1) // SPT
            avail = T - used
            if avail >= tiles_need:
                npad = (-n) % SPT
                ids = order[s:e]
                if npad:
                    ids = np.concatenate([ids, np.full(npad, -1, np.int64)])
                cur_runs.append((c, ids))
                used += tiles_need
                i += 1
            elif avail >= 1:
                take = avail * SPT  # n > take since tiles_need > avail
                cur_runs.append((c, order[s : s + take]))
                used = T
                rem[i] = (c, s + take, e)
                close()
            else:
                close()
            if len(cores) > NCORES:
                return None
        if cur_runs:
            close()
        if len(cores) > NCORES:
            return None
        while len(cores) < NCORES:
            cores.append([])
        return cores

    lo, hi = (N // NCORES) // SPT, ((N // NCORES) // SPT) * 2 + 16
    while lo < hi:
        mid = (lo + hi) // 2
        if pack(mid) is not None:
            hi = mid
        else:
            lo = mid + 1
    NT = ((lo + 3) // 4) * 4  # multiple of OB
    cores = pack(NT)
    assert cores is not None

    # longest-run-first within each core, then pad to NT tiles with the
    # first run's category (slot 0) so the tail is statically indexable too
    padded_ids = []
    tile_cats = []
    run0_tiles = []
    for k in range(NCORES):
        runs = cores[k]
        if not runs:
            runs = [(0, np.full(SPT, -1, np.int64))]
        runs = sorted(runs, key=lambda r: -len(r[1]))
        tcats = []
        parts = []
        for c, ids in runs:
            parts.append(ids)
            tcats.extend([c] * (len(ids) // SPT))
        run0_tiles.append(len(runs[0][1]) // SPT)
        extra = NT - len(tcats)
        if extra:
            parts.append(np.full(extra * SPT, -1, np.int64))
            tcats.extend([runs[0][0]] * extra)
        padded_ids.append(np.concatenate(parts))
        tile_cats.append(tcats)

    SP = min(run0_tiles)
    SP = (SP // OB) * OB  # align static prefix to psum groups

    # per-core weight dedup, first-use order (slot 0 == first run's cat)
    uniq_list = []
    for k in range(NCORES):
        seen = dict()
        for c in tile_cats[k]:
            if c not in seen:
                seen[c] = len(seen)
        uniq_list.append(seen)
    KMAX = max(len(u) for u in uniq_list)

    np_in = _np_in_dtype()
    in_maps = []
    scatter = []
    for k in range(NCORES):
        ids = padded_ids[k]
        valid = ids >= 0
        # gather + zero-pad x rows: [NT*SPT, M, Din]
        Xr = np.zeros((NT * SPT, M, Din), np.float32)
        Xr[valid] = x[ids[valid]]
        # transpose to [Din, NT*P] then chunk the contraction dim
        xT = np.ascontiguousarray(
            Xr.reshape(NT * P, Din).T.astype(np_in)
        ).reshape(2, P, NT * P)

        seen = uniq_list[k]
        w_ids = list(seen.keys())
        w_ids += [w_ids[0]] * (KMAX - len(w_ids))
        Wp = W[np.asarray(w_ids, np.int64)]  # [KMAX, Din, D_H]
        Wl = np.ascontiguousarray(
            Wp.reshape(KMAX, 2, P, D_H).transpose(2, 1, 0, 3).astype(np_in)
        )  # [P, 2, KMAX, D_H]
        W0 = np.ascontiguousarray(Wl[:, :, 0, :])  # [P, 2, D_H] slot-0 copy

        widx = np.asarray([seen[c] for c in tile_cats[k]], np.int32)[None, :]
        assert np.all(widx[0, :SP] == 0)

        in_maps.append({"xT": xT, "Wl": Wl, "W0": W0, "widx": widx})
        scatter.append((ids, valid))

    return in_maps, scatter, NT, KMAX, SP


def _dt_mode():
    return os.environ.get("CSL_DT_MODE", "f16")


def _np_in_dtype():
    import ml_dtypes

    return {
        "f16": np.float16,
        "bf16": ml_dtypes.bfloat16,
        "f32r": np.float32,
        "f32": np.float32,
    }[_dt_mode()]


def _mm_dt():
    return {
        "f16": mybir.dt.float16,
        "bf16": mybir.dt.bfloat16,
        "f32r": mybir.dt.float32r,
        "f32": mybir.dt.float32,
    }[_dt_mode()]


def _build(NT, KMAX, SP):
    """Build the SPMD device program for NT tiles / KMAX slots / SP prefix."""
    mm_dt = _mm_dt()
    out_dt = mybir.dt.float16
    f32 = mybir.dt.float32
    i32 = mybir.dt.int32
    static_idx = os.environ.get("CSL_STATIC", "0") == "1"

    nc = bacc.Bacc(
        "TRN2",
        target_bir_lowering=False,
        debug=False,
        enable_asserts=False,
        num_devices=NCORES,
    )
    NTR = NT * P
    GX = 16  # tiles per x-load DMA group
    xT_d = nc.dram_tensor("xT", [2, P, NTR], mm_dt, kind="ExternalInput").ap()
    W_d = nc.dram_tensor("Wl", [P, 2, KMAX, D_H], mm_dt, kind="ExternalInput").ap()
    W0_d = nc.dram_tensor("W0", [P, 2, D_H], mm_dt, kind="ExternalInput").ap()
    wi_d = nc.dram_tensor("widx", [1, NT], i32, kind="ExternalInput").ap()
    # partition-major output layout: fully contiguous per-partition stores;
    # the host untransposes when scattering back
    out_d = nc.dram_tensor("out", [P, NT, D_H], out_dt, kind="ExternalOutput").ap()

    # store groups with a tapered tail (last two stores small)
    sgroups = []
    t = 0
    while t < NT - OS:
        sgroups.append((t, t + OS))
        t += OS
    rem = NT - t
    if rem > OB:
        sgroups.append((t, t + rem - 2))
        sgroups.append((t + rem - 2, NT))
    else:
        sgroups.append((t, NT))

    with tile.TileContext(nc) as tc:
        with (
            tc.tile_pool(name="wpool", bufs=1) as wpool,
            tc.tile_pool(name="xpool", bufs=5) as xpool,
            tc.tile_pool(name="opool", bufs=3) as opool,
            tc.tile_pool(name="psum", bufs=4, space="PSUM") as psum_pool,
        ):
            # widx first on the Sync ring (tiny, unblocks index loads in the
            # idle startup window); W's c0 half next on Sync while the W0
            # slot-0 copy and the c1 half go on the Act ring — the two W
            # halves stream in parallel and the dynamic tiles' gate (full c0
            # + c1) clears just as the static-prefix tiles finish
            wi_sb = wpool.tile([1, NT], i32)
            nc.sync.dma_start(wi_sb[:], wi_d)
            W0_sb = wpool.tile([P, 2, D_H], mm_dt)
            nc.scalar.dma_start(W0_sb[:], W0_d)
            W_sb = wpool.tile([P, 2, KMAX, D_H], mm_dt)
            nc.sync.dma_start(W_sb[:, 0], W_d[:, 0])
            nc.scalar.dma_start(W_sb[:, 1], W_d[:, 1])

            xts = []  # per-group x tiles on the Sync ring
            for g0 in range(0, NT, GX):
                gx = min(GX, NT - g0)
                xt = xpool.tile([P, 2, GX * P], mm_dt)
                xts.append(xt)
                if g0 == 0:
                    # split the first group so the prefix tiles arrive early
                    h = SP if 0 < SP < gx else gx // 2
                    nc.sync.dma_start(xt[:, 0, : h * P], xT_d[0, :, : h * P])
                    nc.sync.dma_start(xt[:, 1, : h * P], xT_d[1, :, : h * P])
                    nc.sync.dma_start(
                        xt[:, 0, h * P : gx * P], xT_d[0, :, h * P : gx * P]
                    )
                    nc.sync.dma_start(
                        xt[:, 1, h * P : gx * P], xT_d[1, :, h * P : gx * P]
                    )
                else:
                    nc.sync.dma_start(
                        xt[:, 0, : gx * P], xT_d[0, :, g0 * P : (g0 + gx) * P]
                    )
                    nc.sync.dma_start(
                        xt[:, 1, : gx * P], xT_d[1, :, g0 * P : (g0 + gx) * P]
                    )

            # PE index registers for tiles [SP, NT): 16-value loads threaded
            # between early psum groups so each load's sequencer time hides
            # behind matmul work already queued to the PE array
            vals = [0] * NT
            loads = []  # (emit_after_tile, lo, hi)
            lo = SP
            emit_at = min(2 * OB, SP) if SP else 0
            while lo < NT:
                hi = min(lo + VLOAD, NT)
                loads.append((emit_at, lo, hi))
                emit_at += 2 * OB
                lo = hi

            def emit_loads(t_done):
                while loads and loads[0][0] <= t_done:
                    _, llo, lhi = loads.pop(0)
                    if static_idx:
                        continue
                    _, vs = nc.values_load_multi_w_load_instructions(
                        wi_sb[0:1, llo:lhi],
                        engines=(mybir.EngineType.PE,),
                        min_val=0,
                        max_val=KMAX - 1,
                        skip_runtime_bounds_check=True,
                    )
                    for i, v in enumerate(vs):
                        vals[llo + i] = v

            def slot_val(tt):
                return vals[tt]

            if SP == 0:
                emit_loads(0)

            def cast_dve(dst, src):
                nc.vector.tensor_copy(dst, src)

            def cast_act(dst, src):
                nc.scalar.copy(dst, src)

            if os.environ.get("CSL_ACT_CAST", "0") == "1":
                cast_engines = (cast_dve, cast_act)
            else:
                cast_engines = (cast_dve, cast_dve)
            gi = 0
            for g0, g1 in sgroups:
                gsz = g1 - g0
                ot = opool.tile([P, OS, D_H], out_dt)
                o0 = g0
                while o0 < g1:
                    ob_ = min(OB, g1 - o0)
                    ps = psum_pool.tile([P, OB, D_H], f32)
                    for j in range(ob_):
                        tt = o0 + j
                        xg = xts[tt // GX]
                        tl = tt % GX
                        if tt < SP or static_idx:
                            w0c = W0_sb[:, 0:1, :], W0_sb[:, 1:2, :]
                        else:
                            v = slot_val(tt)
                            w0c = (
                                W_sb[:, 0, ds(v, 1), :],
                                W_sb[:, 1, ds(v, 1), :],
                            )
                        nc.tensor.matmul(
                            ps[:, j, :],
                            xg[:, 0, tl * P : (tl + 1) * P],
                            w0c[0],
                            start=True,
                            stop=False,
                        )
                        nc.tensor.matmul(
                            ps[:, j, :],
                            xg[:, 1, tl * P : (tl + 1) * P],
                            w0c[1],
                            start=False,
                            stop=True,
                        )
                    cast = cast_engines[gi % 2]
                    gi += 1
                    cast(ot[:, o0 - g0 : o0 - g0 + ob_], ps[:, :ob_])
                    o0 += ob_
                    emit_loads(o0)
                nc.scalar.dma_start(out_d[:, g0:g1, :], ot[:, :gsz])

    nc.compile()
    return nc


def kernel(x=None, cat_ids=None, W=None, b=None, **_unused):
    global last_results
    x = np.asarray(x, np.float32)
    W = np.asarray(W, np.float32)
    N, M, _ = x.shape

    in_maps, scatter, NT, KMAX, SP = _pack(x, cat_ids, W)

    nc = _build(NT, KMAX, SP)

    trace = os.environ.get("CSL_TRACE", "0") == "1"
    kwargs = {}
    if trace:
        kwargs["trace"] = True
        tc_env = os.environ.get("CSL_TRACE_CORES", "")
        if tc_env:
            kwargs["trace_cores"] = [int(c) for c in tc_env.split(",")]
        else:
            kwargs["trace_cores"] = list(range(NCORES))
    res = run_bass_kernel_spmd(
        nc, in_maps, core_ids=list(range(NCORES)), **kwargs
    )
    last_results = res

    out = np.empty((N, M, D_H), np.float32)
    for k in range(NCORES):
        ids, valid = scatter[k]
        # device layout [P, NT, D_H] -> row-major [NT*P, D_H]
        ok = res.results[k]["out"].astype(np.float32, copy=False)
        ok = ok.transpose(1, 0, 2).reshape(NT * SPT, ROWS_PER_SAMPLE, D_H)
        out[ids[valid]] = ok[valid]

    if b is not None:
        b = np.asarray(b, np.float32)
        if np.any(b):
            cat = np.asarray(cat_ids).astype(np.int64).ravel()
            out += b[cat][:, None, :]

    return out
